# revision 9
# baseline (speedup 1.0000x reference)
"""Trainium2 Bass kernel for nn_LowFreqPenaltyLoss.

Computes mean(|einsum('ih,nchw,jw->ncij', Ch, delta, Cw)|) for
delta [256, 3, 256, 256] f32, Ch/Cw the 8x256 unnormalized DCT-II bases.

Strategy (data-parallel over batch, 8 cores):
  - each core gets 32 batches = 96 images [256, 256] (24 MiB), streamed via
    SWDGE DMAs that cast f32->bf16 inline. Layout per chunk:
    [p, (q e), (r, w)] with image row h = 2p + r, giving 2 KiB contiguous
    f32 per descriptor (half the descriptor count of a row-per-partition
    layout; the stream itself is HBM-bound at ~375 GB/s/core either way).
  - chunk schedule [2, 2, 4, 8 x 10, 4, 2, 2]: the first packets of a DMA
    only drain after its full descriptor set is emitted, so small head
    chunks start the stream ~2 us in instead of ~9; small tail chunks keep
    the post-stream serial pipeline short.
  - the prologue all-engine barrier is stripped: it stalls every engine on
    the slowest engine's boot (~3.3 us) for no correctness benefit here.
  - stage A (contract h): psum[32q+i, (e,w)] += chtp2[:, r, :].T @
    gt[:, 2q:2q+2, r, :] accumulated over r, with the four q col-groups
    packed into one PSUM bank via tile_position.
  - copy bank -> SBUF (ACT, casts to bf16), PE-transpose 128x128 chunks
    (each into its own PSUM bank: transpose-mode output must start at a
    bank boundary on HW), DVE copies out, stage B (contract w):
    out2[(q,i), j] += T.T @ CwT, then fused |.|+sum on DVE into a
    per-partition accumulator.
  - final: ones-matmul partition reduction scaled by 1/49152; host sums
    the 8 per-core partials. bf16 inputs + f32 PSUM accumulation give
    ~2e-4 relative error on the final scalar.
"""

import sys

for _p in ("/root/.axon_site/_ro/trn_rl_repo", "/opt/trn_rl_repo"):
    if _p not in sys.path:
        sys.path.append(_p)

import numpy as np
from contextlib import ExitStack

import concourse.bass as bass
import concourse.tile as tile
from concourse import mybir, bass_utils
from concourse._compat import with_exitstack
from concourse.vector_clock import ScopedClock

# ---------------------------------------------------------------------------
# Workarounds for this image.
# ---------------------------------------------------------------------------

# walrus on this image rejects >1 sync-wait on one CTRL instruction; split the
# Tile exit-drain's waits across follow-up nops (same engine, program order).
# Also: the stock tail (barrier + per-sem clear + barrier) costs ~8-10us of
# EVSEM butterfly at kernel end. The kernel is one-shot per NEFF execution and
# NRT re-initialises semaphores per execution, so keep only the drain + DMA
# completion waits.
_ORIG_DAB = tile.TileContext._drain_and_barrier
_USE_STOCK_TAIL = False


def _patched_drain_and_barrier(self, tick_clock, wait_clock):
    if _USE_STOCK_TAIL:
        return _ORIG_DAB(self, tick_clock, wait_clock)
    nc = self.nc
    drain_inst = nc.sync.drain()
    wait_clock.add_sem_waits(
        drain_inst.ins, ScopedClock({None: tick_clock.global_clock})
    )
    si = drain_inst.ins.sync_info
    waits = list(si.on_wait) if si and si.on_wait else []
    if len(waits) > 1:
        drain_inst.ins.sync_info = mybir.SyncInfo(
            on_wait=[waits[0]], on_update=list(si.on_update or [])
        )
        for w in waits[1:]:
            nop = nc.sync.nop(nofuse=True, hint="drain_wait_split")
            nop.ins.sync_info = mybir.SyncInfo(on_wait=[w], on_update=[])
    popped = nc._tile_sem_poison_stack.pop()
    assert popped is self._sem_poison


tile.TileContext._drain_and_barrier = _patched_drain_and_barrier

# zero-egress container: profiling artifact upload must stay local.
bass_utils.upload_artifacts = lambda d: d


def _strip_main_barrier(nc):
    """Drop the prologue all-engine barrier in 'main': its only role is to
    fence the framework preamble (dead const memsets + per-engine table
    loads) from the kernel, but per-engine program order already covers the
    table loads and nothing reads the const tiles. The barrier otherwise
    stalls every engine (including the DMA-issuing ones) on the slowest
    engine's multi-microsecond boot."""
    for fn in nc.m.functions:
        for bb in fn.blocks:
            if bb.name != "main":
                continue
            bb.instructions[:] = [
                i for i in bb.instructions
                if not isinstance(i, (mybir.InstEventSemaphore, mybir.InstDrain,
                                      mybir.InstMemset))
            ]


def _split_multi_waits(nc):
    """walrus on this image rejects >1 sync-wait per instruction: hoist extra
    waits onto fresh NoOps inserted just before, on the same engine."""
    for fn in nc.m.functions:
        for bb in fn.blocks:
            new_insts = []
            for inst in bb.instructions:
                si = inst.sync_info
                waits = list(si.on_wait) if si and si.on_wait else []
                if len(waits) > 1:
                    for w in waits[:-1]:
                        nop = mybir.InstNoOp(
                            name=nc.get_next_instruction_name(),
                            sync_info=mybir.SyncInfo(on_wait=[w], on_update=[]),
                            bass_nofuse=True,
                            engine=inst.engine,
                        )
                        new_insts.append(nop)
                    inst.sync_info = mybir.SyncInfo(
                        on_wait=[waits[-1]], on_update=list(si.on_update or [])
                    )
                new_insts.append(inst)
            bb.instructions[:] = new_insts

# ---------------------------------------------------------------------------
# Problem constants (hardcoded; kernel.py must be self-contained).
# ---------------------------------------------------------------------------

B, C, H, W = 256, 3, 256, 256
LOW_A = LOW_B = 8
N_CORES = 8
IMGS_PER_CORE = (B // N_CORES) * C          # 96
TOTAL_LOW = B * C * LOW_A * LOW_B           # 49152 -> mean divisor

# image chunks per core: small head chunks so the stream's first packets
# drain early (a DMA's first packet waits on its full descriptor emission),
# small tail chunks so the post-stream serial pipeline is shallow.
CHUNKS = [2, 2, 4] + [8] * 10 + [4, 2, 2]
assert sum(CHUNKS) == IMGS_PER_CORE

F32 = mybir.dt.float32
BF16 = mybir.dt.bfloat16


def _dct_basis(K, N):
    n = np.arange(N, dtype=np.float64)
    k = np.arange(K, dtype=np.float64)
    return (2.0 * np.cos(np.pi * (2.0 * n[None, :] + 1.0) * k[:, None] / (2.0 * N))).astype(
        np.float32
    )


def _make_consts():
    Ch = _dct_basis(LOW_A, H)   # [8, 256]
    Cw = _dct_basis(LOW_B, W)   # [8, 256]
    # chtp2[p, r, i] = Ch[i, 2p+r], padded to 32 cols with zeros so the full
    # 32-wide PE col-group is written (garbage-free bank).
    chtp2 = np.zeros((128, 2, 32), np.float32)
    for r in range(2):
        chtp2[:, r, :8] = Ch[:, r::2].T   # Ch[i, 2p+r] -> [p, i]
    # cwt[p, wc, j] = Cw[j, wc*128+p]
    cwt = np.zeros((128, 2, 8), np.float32)
    for wc in range(2):
        cwt[:, wc, :] = Cw[:, wc * 128:(wc + 1) * 128].T
    import ml_dtypes
    bf16 = ml_dtypes.bfloat16
    ident = np.eye(128, dtype=bf16)
    sumw = np.full((128, 1), 1.0 / TOTAL_LOW, np.float32)
    return chtp2.astype(bf16), cwt.astype(bf16), ident, sumw


CHTP2, CWT, IDENT, SUMW = _make_consts()


# ---------------------------------------------------------------------------
# Kernel body (per core; SPMD over 8 cores).
# ---------------------------------------------------------------------------

@with_exitstack
def _lowfreq_kernel(ctx: ExitStack, tc, out_ap, delta_ap, chtp_ap, cwt_ap,
                    ident_ap, sumw_ap):
    nc = tc.nc

    const_pool = ctx.enter_context(tc.tile_pool(name="const", bufs=1))
    in_pool = ctx.enter_context(tc.tile_pool(name="input", bufs=10))
    edge_pool = ctx.enter_context(tc.tile_pool(name="edge", bufs=4))
    sS_pool = ctx.enter_context(tc.tile_pool(name="sS", bufs=4))
    tS_pool = ctx.enter_context(tc.tile_pool(name="tS", bufs=3))
    red_pool = ctx.enter_context(tc.tile_pool(name="red", bufs=2))
    acc_pool = ctx.enter_context(tc.tile_pool(name="acc", bufs=1))
    psA_pool = ctx.enter_context(tc.tile_pool(name="psA", bufs=3, space="PSUM"))
    psT_pool = ctx.enter_context(tc.tile_pool(name="psT", bufs=3, space="PSUM"))
    ps2_pool = ctx.enter_context(tc.tile_pool(name="ps2", bufs=2, space="PSUM"))

    # constants first on the sync (HWDGE) ring: tiny, land well before use.
    chtp2 = const_pool.tile([128, 2, 32], BF16)     # [p, r, i]
    nc.sync.dma_start(chtp2[:], chtp_ap)
    cwt = const_pool.tile([128, 2, 8], BF16)        # [p, wc, j]
    nc.sync.dma_start(cwt[:], cwt_ap)
    ident = const_pool.tile([128, 128], BF16)
    nc.sync.dma_start(ident[:], ident_ap)
    sumw = const_pool.tile([128, 1], F32)
    nc.sync.dma_start(sumw[:], sumw_ap)

    # input chunk tiles + their SWDGE cast-DMAs (f32 -> bf16 inline), all
    # issued up front in schedule order. Layout [p, (q e), r, w] with image
    # row h = 2p + r: 2 KiB contiguous f32 source per descriptor.
    gts = []
    img0 = 0
    for n_img in CHUNKS:
        pool = in_pool if n_img == 8 else edge_pool
        gt = pool.tile([128, n_img, 2, 256], BF16,
                       tag=None if n_img == 8 else f"edge{n_img}")
        src = delta_ap[img0:img0 + n_img, :, :]
        nc.gpsimd.dma_start(
            gt[:],
            src.rearrange("m (p r) w -> p m (r w)", p=128, r=2),
        )
        gts.append(gt)
        img0 += n_img

    acc = acc_pool.tile([128, 1], F32)
    nc.vector.memset(acc[:], 0.0)

    # Software-pipelined emission: the PE queue is FIFO, so a transpose
    # waiting on ACT (or a stage-B matmul waiting on DVE) would stall the
    # next chunk's independent stage-A matmuls behind it and keep the HAM
    # clock-gate throttled. Emit A_i, then tp_{i-1}, then B_{i-2}: every PE
    # instruction's dependencies resolved roughly a chunk-time ago.
    n_chunk = len(CHUNKS)
    sS_of = {}
    tps_of = {}
    tSb_of = {}

    def emit_stageA(i):
        gt, n_img = gts[i], CHUNKS[i]
        npairs = n_img // 2
        nrows = 32 * npairs
        bankA = psA_pool.tile([128, 512], F32, tag="bankA")
        for r in range(2):
            for q in range(npairs):
                nc.tensor.matmul(
                    bankA[32 * q:32 * q + 32, :],
                    lhsT=chtp2[:, r, :],
                    rhs=gt[:, 2 * q:2 * q + 2, r, :],
                    start=(r == 0),
                    stop=(r == 1),
                    tile_position=(0, 32 * q),
                    # CoreSim's zero-region tracker is bank-granular and
                    # flags the concurrent per-partition col-groups; HW
                    # has_written state is per-element.
                    skip_group_check=True,
                )
        # PSUM -> SBUF with f32->bf16 cast (ACT engine)
        sS = sS_pool.tile([128, 512], BF16, tag="sS")
        nc.scalar.copy(sS[:nrows, :], bankA[:nrows, :])
        sS_of[i] = sS

    def emit_tp(i):
        nrows = 32 * (CHUNKS[i] // 2)
        sS = sS_of.pop(i)
        tps = []
        for c in range(4):
            tp = psT_pool.tile([128, 128], BF16, tag="tp")
            nc.tensor.transpose(
                tp[:, :nrows],
                sS[:nrows, 128 * c:128 * c + 128],
                ident[:nrows, :nrows],
            )
            tps.append(tp)
        tSb = tS_pool.tile([128, 512], BF16, tag="tSb")
        for c in range(4):
            nc.vector.tensor_copy(tSb[:, 128 * c:128 * c + nrows], tps[c][:, :nrows])
        tSb_of[i] = tSb

    def emit_stageB(i):
        nrows = 32 * (CHUNKS[i] // 2)
        tSb = tSb_of.pop(i)
        ps2 = ps2_pool.tile([128, 16], F32, tag="ps2")
        for e in range(2):
            for wc in range(2):
                c = 2 * e + wc
                nc.tensor.matmul(
                    ps2[:nrows, 8 * e:8 * e + 8],
                    lhsT=tSb[:, 128 * c:128 * c + nrows],
                    rhs=cwt[:, wc, :],
                    start=(wc == 0),
                    stop=(wc == 1),
                )
        red = red_pool.tile([128, 1], F32, tag="red")
        nc.vector.tensor_reduce(
            red[:nrows], ps2[:nrows, :], axis=mybir.AxisListType.X,
            op=mybir.AluOpType.add, apply_absolute_value=True,
        )
        nc.vector.tensor_add(acc[:nrows], acc[:nrows], red[:nrows])

    for i in range(n_chunk + 3):
        if i < n_chunk:
            emit_stageA(i)
        if 0 <= i - 2 < n_chunk:
            emit_tp(i - 2)
        if 0 <= i - 3 < n_chunk:
            emit_stageB(i - 3)

    # final partition reduction: out = acc.T @ sumw = sum_p acc[p] / 49152
    fout = ps2_pool.tile([1, 1], F32, tag="ps2")
    nc.tensor.matmul(fout[:], lhsT=acc[:], rhs=sumw[:], start=True, stop=True)
    fsb = red_pool.tile([1, 1], F32, tag="fsb")
    nc.vector.tensor_copy(fsb[:], fout[:])
    nc.sync.dma_start(out_ap, fsb[:])


# ---------------------------------------------------------------------------
# Build + run.
# ---------------------------------------------------------------------------

_CACHED_NC = None


def _build(for_sim=False):
    global _CACHED_NC, _USE_STOCK_TAIL
    if not for_sim and _CACHED_NC is not None:
        return _CACHED_NC
    _USE_STOCK_TAIL = for_sim
    nc = bass.Bass("TRN2", target_bir_lowering=False, debug=False)
    delta = nc.dram_tensor("delta", [IMGS_PER_CORE, H, W], F32, kind="ExternalInput")
    chtp = nc.dram_tensor("chtp", list(CHTP2.shape), BF16, kind="ExternalInput")
    cwt = nc.dram_tensor("cwt", list(CWT.shape), BF16, kind="ExternalInput")
    ident = nc.dram_tensor("ident", list(IDENT.shape), BF16, kind="ExternalInput")
    sumw = nc.dram_tensor("sumw", list(SUMW.shape), F32, kind="ExternalInput")
    out = nc.dram_tensor("out", [1, 1], F32, kind="ExternalOutput")

    with tile.TileContext(nc) as tc:
        _lowfreq_kernel(
            tc, out.ap(), delta.ap(), chtp.ap(), cwt.ap(), ident.ap(), sumw.ap()
        )
    _USE_STOCK_TAIL = False
    if for_sim:
        return nc
    _strip_main_barrier(nc)
    _split_multi_waits(nc)
    _CACHED_NC = nc
    return nc


def _run(delta, **spmd_kwargs):
    import os
    os.environ["JAX_PLATFORMS"] = "axon"   # harness may have pinned cpu for the reference
    nc = _build()
    delta = np.ascontiguousarray(np.asarray(delta, dtype=np.float32))
    assert delta.shape == (B, C, H, W)
    shards = delta.reshape(N_CORES, IMGS_PER_CORE, H, W)
    in_maps = [
        {
            "delta": shards[i],
            "chtp": CHTP2,
            "cwt": CWT,
            "ident": IDENT,
            "sumw": SUMW,
        }
        for i in range(N_CORES)
    ]
    try:
        res = bass_utils.run_bass_kernel_spmd(
            nc, in_maps, core_ids=list(range(N_CORES)), **spmd_kwargs
        )
    except Exception:
        # transient NRT_EXEC_UNIT_UNRECOVERABLE has been observed on this
        # terminal; one retry typically succeeds.
        res = bass_utils.run_bass_kernel_spmd(
            nc, in_maps, core_ids=list(range(N_CORES)), **spmd_kwargs
        )
    total = np.float64(0.0)
    for r in res.results:
        total += np.float64(r["out"][0, 0])
    return np.float32(total).reshape(()), res


def kernel(delta):
    out, _ = _run(delta)
    return out
